# revision 2
# baseline (speedup 1.0000x reference)
"""BailingMoE block on 8 Trainium2 NeuronCores — v11.

Sharding:
  - Attention: HEAD-parallel. Core c owns q-heads {2c, 2c+1} and kv-head
    c//2 over ALL 1024 tokens, fed by a host-pretransposed, host-normalized
    h1^T (input rmsnorm is pure input prep).  Causal structure is identical
    on every core so upper-triangle score tiles are statically skipped.
    No kv AllGather.
  - attn_out partials are ReduceScatter-summed (bf16, split into two
    H-halves so the second half's wo overlap the first RS); core c gets its
    own 128-token chunk; residual/rmsnorm/fp32 router run locally.
  - MoE: 2 token-groups x 4 expert-shards; core c computes experts
    {2(c%4), 2(c%4)+1} over its group's 512 tokens.  h2 goes out fp8 via a
    4-core AllGather; expert outputs ReduceScatter back (bf16, 2 halves).
  - Expert matmuls run fp8e4 DoubleRow with weight-compensation:
    W ~ q8(W) + q8(16(W-q8(W)))/16, the 1/16 folded into a scaled copy of
    the activations (h2/16, act/16).  Intermediate activations stay bf16;
    the shared expert is fully bf16.  Router fp32 (flip-safe).
"""

import numpy as np

import concourse.bass as bass
import concourse.bacc as bacc
import concourse.mybir as mybir
import concourse.tile as tile
from concourse.bass_utils import run_bass_kernel_spmd
from concourse.masks import make_identity

F32 = mybir.dt.float32
BF16 = mybir.dt.bfloat16
F8 = mybir.dt.float8e4
AF = mybir.ActivationFunctionType
ALU = mybir.AluOpType
AX = mybir.AxisListType
DR = mybir.MatmulPerfMode.DoubleRow

N_CORES = 8
T = 1024
TC = 128
TG = 512
H = 2048
KH = H // 128
NH = 16
NKV = 4
DH = 128
E = 8
I = 1024
IS = 1024
EPS = 1e-6
SCALE = DH ** -0.5
NEG = -30000.0

NT = T // TC
NTG = TG // TC

_cache = {}


def _bc(ap, n, axis=1):
    a = [list(p) for p in ap.ap]
    a.insert(axis, [0, n])
    return bass.AP(tensor=ap.tensor, offset=ap.offset, ap=a)


def build_nc():
    nc = bacc.Bacc("TRN2", target_bir_lowering=False, num_devices=N_CORES)

    h1T_in = nc.dram_tensor("h1T_in", [KH, 128, T], BF16, kind="ExternalInput")
    wqkv_sl = nc.dram_tensor("wqkv_sl", [KH, 128, 512], BF16,
                             kind="ExternalInput")
    wo_sl = nc.dram_tensor("wo_sl", [2, 128, H], BF16, kind="ExternalInput")
    x_own = nc.dram_tensor("x_own", [TC, H], F32, kind="ExternalInput")
    rope_q = nc.dram_tensor("rope_q", [128, NT * 4 * 64], F32,
                            kind="ExternalInput")
    rope_k = nc.dram_tensor("rope_k", [128, NT * 4 * 64], F32,
                            kind="ExternalInput")
    trimask = nc.dram_tensor("trimask", [128, 128], F32, kind="ExternalInput")
    wrT = nc.dram_tensor("wrT", [KH, 128, E], F32, kind="ExternalInput")
    # routed experts: fp8 hi/lo packs (lo pre-scaled by 16)
    wgu_pk = nc.dram_tensor("wgu_pk", [2, 2, I // 128, 128, 4096], F8,
                            kind="ExternalInput")
    wd_pk = nc.dram_tensor("wd_pk", [2, 2, 128, 16384], F8,
                           kind="ExternalInput")
    # shared expert: bf16 packs
    wsgu_pk = nc.dram_tensor("wsgu_pk", [2 * IS // 128, 128, KH, 128], BF16,
                             kind="ExternalInput")
    wsd_pk = nc.dram_tensor("wsd_pk", [IS // 128, 128, H], BF16,
                            kind="ExternalInput")
    esel_bc = nc.dram_tensor("esel_bc", [E, 2, 128], BF16,
                             kind="ExternalInput")
    out_chunk = nc.dram_tensor("out_chunk", [TC, H], F32,
                               kind="ExternalOutput")

    rg8 = [list(range(N_CORES))]
    rg4 = [[0, 1, 2, 3], [4, 5, 6, 7]]

    with tile.TileContext(nc) as tc:
        with tc.tile_pool(name="dram", bufs=1, space="DRAM") as dram, \
             tc.tile_pool(name="const", bufs=1) as const, \
             tc.tile_pool(name="mid", bufs=1) as mid, \
             tc.tile_pool(name="sb", bufs=2) as sb, \
             tc.tile_pool(name="pfw", bufs=1) as pfw, \
             tc.tile_pool(name="ps_big", bufs=4, space="PSUM") as ps_big, \
             tc.tile_pool(name="ps_sm", bufs=2, space="PSUM") as ps_sm:

            # ---- DRAM collective buffers (per-half RS for attn and moe) ----
            attn_in = dram.tile([T, H], BF16)
            attn_out = dram.tile([TC, H], BF16)
            hag_in = dram.tile([H, TC], F8)
            hag_out = dram.tile([NTG * H, TC], F8)
            wag_in = dram.tile([32, TC], F8)
            wag_out = dram.tile([NTG * 32, TC], F8)
            moe_in = [dram.tile([TG, H // 2], BF16, name=f"moe_in{q}")
                      for q in range(2)]
            moe_out = [dram.tile([TC, H // 2], BF16, name=f"moe_out{q}")
                       for q in range(2)]

            # ---- constants ----
            ident_bf = const.tile([128, 128], BF16)
            make_identity(nc, ident_bf)
            ident_f = const.tile([128, 128], F32)
            make_identity(nc, ident_f)
            ones_row = const.tile([1, 128], F32)
            nc.vector.memset(ones_row, 1.0)
            eps_sb = const.tile([128, 1], F32)
            nc.vector.memset(eps_sb, EPS)
            c16_sb = const.tile([128, 1], F32)
            nc.vector.memset(c16_sb, 1.0 / 16.0)
            tri_sb = const.tile([128, 128], F32)
            nc.sync.dma_start(out=tri_sb, in_=trimask[:, :])
            esel_sb = const.tile([E, 2, 128], BF16)
            nc.sync.dma_start(
                out=esel_sb,
                in_=bass.AP(tensor=esel_bc, offset=0,
                            ap=[[256, E], [128, 2], [1, 128]]))
            wrT_sb = const.tile([128, KH, E], F32)
            nc.sync.dma_start(
                out=wrT_sb,
                in_=bass.AP(tensor=wrT, offset=0,
                            ap=[[E, 128], [128 * E, KH], [1, E]]))

            # ---- persistent tiles ----
            x2_sb = mid.tile([TC, H], F32)
            shared_sb = mid.tile([TC, H], F32)
            h2T_own8 = mid.tile([128, KH, TC], F8)
            h2T_ownb = mid.tile([128, KH, TC], BF16)
            rs2_col = mid.tile([TC, 1], F32)

            # ================= ATTENTION =================
            with tc.tile_pool(name="ap_", bufs=2) as ap_:
                wqkv_sb = ap_.tile([128, KH, 512], BF16, tag="wqkv", bufs=1)
                h1T = [None] * KH

                def _load_h1T(k):
                    t_ = ap_.tile([128, T], BF16, tag=f"h1T{k}", bufs=1,
                                  name=f"h1T{k}")
                    nc.sync.dma_start(out=t_, in_=h1T_in[k, :, :])
                    h1T[k] = t_

                def _load_wqkv(kq):
                    nc.sync.dma_start(
                        out=wqkv_sb[:, 4 * kq:4 * (kq + 1), :],
                        in_=bass.AP(tensor=wqkv_sl,
                                    offset=4 * kq * 128 * 512,
                                    ap=[[512, 128], [128 * 512, 4],
                                        [1, 512]]))

                _load_wqkv(0)
                _load_h1T(0)
                _load_h1T(1)
                _load_wqkv(1)
                _load_h1T(2)
                _load_h1T(3)
                _load_wqkv(2)
                _load_wqkv(3)
                for k in range(4, KH):
                    _load_h1T(k)
                rq_f = ap_.tile([128, NT * 256], F32, tag="ropeq", bufs=1)
                nc.sync.dma_start(
                    out=rq_f,
                    in_=bass.AP(tensor=rope_q, offset=0,
                                ap=[[NT * 256, 128], [1, NT * 256]]))
                rk_f = ap_.tile([128, NT * 256], F32, tag="ropek", bufs=1)
                nc.sync.dma_start(
                    out=rk_f,
                    in_=bass.AP(tensor=rope_k, offset=0,
                                ap=[[NT * 256, 128], [1, NT * 256]]))
                rope_q_sb = rq_f.rearrange("p (t f c) -> p t f c", t=NT, f=4)
                rope_k_sb = rk_f.rearrange("p (t f c) -> p t f c", t=NT, f=4)

                # prefetch wd-hi into a persistent pool during attention
                wdhi = [pfw.tile([128, 16384], F8, tag=f"wdhi{e}", bufs=1,
                                 name=f"wdhi{e}") for e in range(2)]
                for e in range(2):
                    nc.sync.dma_start(out=wdhi[e], in_=wd_pk[e, 0, :, :])

                kT_sb = ap_.tile([128, NT, 128], BF16, tag="kT", bufs=1)
                v_sb = [ap_.tile([128, DH + 1], BF16, tag=f"v{t}", bufs=1,
                                 name=f"v{t}") for t in range(NT)]
                qT_sb = [ap_.tile([128, NT, 128], BF16, tag=f"qT{h}", bufs=1,
                                  name=f"qT{h}") for h in range(2)]

                def rope(x3, obf3, nh, tab, t):
                    c1 = _bc(tab[:, t, 0, :], nh)
                    s1 = _bc(tab[:, t, 1, :], nh)
                    c2 = _bc(tab[:, t, 2, :], nh)
                    s2 = _bc(tab[:, t, 3, :], nh)
                    x1 = x3[:, :, 0:64]
                    x2 = x3[:, :, 64:128]
                    t1 = ap_.tile([TC, 2, 64], F32, tag="rp1")
                    tn = ap_.tile([TC, 2, 64], F32, tag="rpn")
                    t1v = t1[:, :nh, :]
                    tnv = tn[:, :nh, :]
                    nc.vector.tensor_mul(t1v, x1, c1)
                    nc.vector.tensor_mul(tnv, x2, s1)
                    nc.vector.tensor_sub(t1v, t1v, tnv)
                    nc.vector.tensor_copy(obf3[:, :, 0:64], t1v)
                    nc.vector.tensor_mul(t1v, x2, c2)
                    nc.vector.tensor_mul(tnv, x1, s2)
                    nc.vector.tensor_add(t1v, t1v, tnv)
                    nc.vector.tensor_copy(obf3[:, :, 64:128], t1v)

                # qkv in token-groups of 3 (k-inner) for early PE start
                for tgrp in ([0, 1, 2], [3, 4, 5], [6, 7]):
                    pqs = []
                    for ti in range(len(tgrp)):
                        pq_t = ps_big.tile([TC, 512], F32, tag="mm512",
                                           name=f"psqkv{ti}")
                        pqs.append(pq_t)
                    for k in range(KH):
                        for ti, t in enumerate(tgrp):
                            nc.tensor.matmul(
                                pqs[ti], h1T[k][:, t * TC:(t + 1) * TC],
                                wqkv_sb[:, k, :],
                                start=(k == 0), stop=(k == KH - 1))
                    for ti, t in enumerate(tgrp):
                        pq = pqs[ti]
                        qk_f = ap_.tile([TC, 384], F32, tag="qkf")
                        nc.vector.tensor_copy(qk_f, pq[:, 0:384])
                        sq = ap_.tile([TC, 384], F32, tag="qksq")
                        nc.vector.tensor_mul(sq, qk_f, qk_f)
                        red = ap_.tile([TC, 3, 1], F32, tag="qkred")
                        nc.vector.tensor_reduce(
                            red, sq.rearrange("p (h d) -> p h d", h=3),
                            axis=AX.X, op=ALU.add)
                        red2 = red.rearrange("p h one -> p (h one)")
                        nc.scalar.activation(red2, red2, AF.Sqrt,
                                             bias=eps_sb[:TC], scale=1.0 / DH)
                        nc.vector.reciprocal(red2, red2)
                        for h in range(3):
                            nc.vector.tensor_scalar_mul(
                                qk_f[:, h * DH:(h + 1) * DH],
                                qk_f[:, h * DH:(h + 1) * DH], red[:, h, :])
                        qbf = ap_.tile([TC, 2, DH], BF16, tag="qbf")
                        rope(qk_f[:, 0:256].rearrange("p (h d) -> p h d", h=2),
                             qbf, 2, rope_q_sb, t)
                        kbf = ap_.tile([TC, 1, DH], BF16, tag="kbf")
                        rope(qk_f[:, 256:384].rearrange("p (h d) -> p h d",
                                                        h=1),
                             kbf, 1, rope_k_sb, t)
                        nc.scalar.activation(v_sb[t][:, 0:DH], pq[:, 384:512],
                                             AF.Copy)
                        nc.vector.memset(v_sb[t][:, DH:DH + 1], 1.0)
                        pt = ps_sm.tile([128, 128], BF16, tag="pstb")
                        nc.tensor.transpose(pt, kbf[:, 0, :], ident_bf)
                        nc.vector.tensor_copy(kT_sb[:, t, :], pt)
                        for h in range(2):
                            pt2 = ps_sm.tile([128, 128], BF16, tag="pstb")
                            nc.tensor.transpose(pt2, qbf[:, h, :], ident_bf)
                            nc.vector.tensor_copy(qT_sb[h][:, t, :], pt2)

                wo_sb = ap_.tile([128, 2, H], BF16, tag="wo", bufs=1)
                nc.sync.dma_start(
                    out=wo_sb,
                    in_=bass.AP(tensor=wo_sl, offset=0,
                                ap=[[H, 128], [128 * H, 2], [1, H]]))
                ctxT_sb = [ap_.tile([128, NT, 128], BF16, tag=f"ctxT{h}",
                                    bufs=1, name=f"ctxT{h}")
                           for h in range(2)]
                probs2 = [[None] * NT, [None] * NT]
                for h in range(2):
                    probs = probs2[h]
                    for kt in range(NT):
                        width = T - kt * TC
                        pb = ap_.tile([128, width], BF16, tag=f"probs{kt}",
                                     bufs=2, name=f"probs{kt}")
                        probs[kt] = pb
                        off = 0
                        while off < width:
                            w = min(512, width - off)
                            ps_s = ps_big.tile([TC, 512], F32, tag="mm512")
                            nc.tensor.matmul(
                                ps_s[:, 0:w], kT_sb[:, kt, :],
                                qT_sb[h].rearrange("p t q -> p (t q)")
                                [:, kt * TC + off: kt * TC + off + w],
                                start=True, stop=True)
                            if off == 0:
                                nc.vector.tensor_add(ps_s[:, 0:128],
                                                     ps_s[:, 0:128], tri_sb)
                            nc.scalar.activation(pb[:, off:off + w],
                                                 ps_s[:, 0:w], AF.Exp,
                                                 scale=SCALE)
                            off += w
                for h in range(2):
                    probs = probs2[h]
                    for t in range(NT):
                        pctx = ps_sm.tile([TC, DH + 1], F32, tag="pctx",
                                          bufs=2)
                        for kt in range(t + 1):
                            nc.tensor.matmul(
                                pctx,
                                probs[kt][:, (t - kt) * TC:(t - kt + 1) * TC],
                                v_sb[kt], start=(kt == 0), stop=(kt == t))
                        rden = ap_.tile([TC, 1], F32, tag="rden")
                        nc.vector.reciprocal(rden, pctx[:, DH:DH + 1])
                        ctx_bf = ap_.tile([TC, DH], BF16, tag="ctxbf")
                        nc.vector.tensor_scalar_mul(ctx_bf, pctx[:, 0:DH],
                                                    rden)
                        pt = ps_sm.tile([128, 128], BF16, tag="pstb")
                        nc.tensor.transpose(pt, ctx_bf, ident_bf)
                        nc.vector.tensor_copy(ctxT_sb[h][:, t, :], pt)

                # wo partials -> [T, H] -> single ReduceScatter
                for t in range(NT):
                    attn_bf = ap_.tile([TC, H], BF16, tag="attnbf")
                    for n in range(H // 512):
                        po = ps_big.tile([TC, 512], F32, tag="mm512")
                        for h in range(2):
                            nc.tensor.matmul(
                                po, ctxT_sb[h][:, t, :],
                                wo_sb[:, h, n * 512:(n + 1) * 512],
                                start=(h == 0), stop=(h == 1))
                        nc.scalar.activation(
                            attn_bf[:, n * 512:(n + 1) * 512], po, AF.Copy)
                    nc.scalar.dma_start(
                        out=attn_in[t * TC:(t + 1) * TC, :], in_=attn_bf)
                nc.gpsimd.collective_compute(
                    "ReduceScatter", ALU.add, replica_groups=rg8,
                    ins=[attn_in.opt()], outs=[attn_out.opt()])

            # ================= x2 + h2 + ROUTER (own chunk) =================
            with tc.tile_pool(name="rp", bufs=2) as rp:
                x_sb = rp.tile([TC, H], F32, tag="xown", bufs=1)
                nc.sync.dma_start(out=x_sb, in_=x_own[:, :])
                ao_sb = rp.tile([TC, H], BF16, tag="aors", bufs=1)
                nc.scalar.dma_start(out=ao_sb, in_=attn_out[:, :])
                nc.vector.tensor_add(x2_sb, x_sb, ao_sb)
                sq2 = rp.tile([TC, H], F32, tag="sq2", bufs=1)
                nc.scalar.activation(sq2, x2_sb, AF.Square,
                                     accum_out=rs2_col)
                nc.scalar.activation(rs2_col, rs2_col, AF.Sqrt,
                                     bias=eps_sb[:TC], scale=1.0 / H)
                nc.vector.reciprocal(rs2_col, rs2_col)

                # h2 (own chunk): bf16 + fp8 transposed copies, AG first
                h2_bf = rp.tile([TC, H], BF16, tag="h2bf", bufs=1)
                nc.vector.tensor_scalar_mul(h2_bf, x2_sb, rs2_col)
                for j in range(KH):
                    pt = ps_sm.tile([128, 128], BF16, tag="pstb")
                    nc.tensor.transpose(pt, h2_bf[:, j * 128:(j + 1) * 128],
                                        ident_bf)
                    nc.scalar.activation(h2T_own8[:, j, :], pt, AF.Copy)
                    nc.vector.tensor_copy(h2T_ownb[:, j, :], pt)
                nc.scalar.dma_start(
                    out=bass.AP(tensor=hag_in.tensor, offset=hag_in.offset,
                                ap=[[TC, 128], [128 * TC, KH], [1, TC]]),
                    in_=h2T_own8)
                nc.gpsimd.collective_compute(
                    "AllGather", ALU.bypass, replica_groups=rg4,
                    ins=[hag_in.opt()], outs=[hag_out.opt()])

                # fp32 router (flip-safe): logits = (x2 @ wrT) * rs2
                pr = ps_big.tile([TC, 512], F32, tag="mm512")
                for j in range(KH):
                    ptf = ps_sm.tile([128, 128], F32, tag="pstb")
                    nc.tensor.transpose(ptf, x2_sb[:, j * 128:(j + 1) * 128],
                                        ident_f)
                    x2T_j = rp.tile([128, TC], F32, tag="x2Tj")
                    nc.vector.tensor_copy(x2T_j, ptf)
                    nc.tensor.matmul(pr[:, 0:E], x2T_j, wrT_sb[:, j, :],
                                     start=(j == 0), stop=(j == KH - 1))

                logits = rp.tile([TC, E], F32, tag="logits", bufs=1)
                nc.vector.tensor_scalar_mul(logits, pr[:, 0:E], rs2_col)
                probs8 = rp.tile([TC, E], F32, tag="probs8", bufs=1)
                nc.scalar.activation(probs8, logits, AF.Exp, scale=1.0)
                den8 = rp.tile([TC, 1], F32, tag="den8")
                nc.vector.tensor_reduce(den8, probs8, axis=AX.X, op=ALU.add)
                rden8 = rp.tile([TC, 1], F32, tag="rden8")
                nc.vector.reciprocal(rden8, den8)
                nc.vector.tensor_scalar_mul(probs8, probs8, rden8)
                mx8 = rp.tile([TC, 8], F32, tag="mx8")
                nc.vector.max(out=mx8, in_=probs8)
                s12 = rp.tile([TC, 1], F32, tag="s12")
                nc.vector.tensor_add(s12, mx8[:, 0:1], mx8[:, 1:2])
                rs12 = rp.tile([TC, 1], F32, tag="rs12")
                nc.vector.reciprocal(rs12, s12)
                eq1 = rp.tile([TC, E], F32, tag="eq1")
                nc.vector.tensor_scalar(eq1, probs8, mx8[:, 0:1], None,
                                        op0=ALU.is_equal)
                eq2 = rp.tile([TC, E], F32, tag="eq2")
                nc.vector.tensor_scalar(eq2, probs8, mx8[:, 1:2], None,
                                        op0=ALU.is_equal)
                nc.vector.tensor_add(eq1, eq1, eq2)
                wm = rp.tile([TC, E], F32, tag="wm", bufs=1)
                nc.vector.tensor_mul(wm, probs8, eq1)
                nc.vector.tensor_scalar_mul(wm, wm, rs12)
                # transpose + fp8 hi/lo encode into the h2 AG payload
                ptw = ps_sm.tile([E, TC], F32, tag="pctx")
                nc.tensor.transpose(ptw, wm, ident_f)
                whi8_s = rp.tile([E, TC], F8, tag="whi8s", bufs=1)
                nc.scalar.activation(whi8_s, ptw, AF.Copy)
                hi_f = rp.tile([E, TC], F32, tag="hif", bufs=1)
                nc.vector.tensor_copy(hi_f, whi8_s)
                lo_f = rp.tile([E, TC], F32, tag="lof", bufs=1)
                nc.vector.tensor_sub(lo_f, ptw, hi_f)
                wlo8_s = rp.tile([E, TC], F8, tag="wlo8s", bufs=1)
                nc.scalar.activation(wlo8_s, lo_f, AF.Copy, scale=16.0)
                nc.scalar.dma_start(out=wag_in[0:E, :], in_=whi8_s)
                nc.scalar.dma_start(out=wag_in[E:16, :], in_=wlo8_s)
                nc.gpsimd.collective_compute(
                    "AllGather", ALU.bypass, replica_groups=rg4,
                    ins=[wag_in.opt()], outs=[wag_out.opt()])

            # ================= SHARED EXPERT (own chunk, bf16) ========
            with tc.tile_pool(name="shp", bufs=2) as shp:
                acts_s = shp.tile([128, IS // 128, TC], BF16, tag="actss",
                                  bufs=1)
                for i in range(IS // 128):
                    # g tile then u tile (m-tiles 2i, 2i+1 = g_i, u_i)
                    wsg_f = shp.tile([128, KH * 128], BF16, tag="wsg")
                    nc.sync.dma_start(
                        out=wsg_f,
                        in_=bass.AP(tensor=wsgu_pk,
                                    offset=2 * i * 128 * KH * 128,
                                    ap=[[KH * 128, 128], [1, KH * 128]]))
                    wsu_f = shp.tile([128, KH * 128], BF16, tag="wsu")
                    nc.sync.dma_start(
                        out=wsu_f,
                        in_=bass.AP(tensor=wsgu_pk,
                                    offset=(2 * i + 1) * 128 * KH * 128,
                                    ap=[[KH * 128, 128], [1, KH * 128]]))
                    wsg = wsg_f.rearrange("p (k m) -> p k m", k=KH)
                    wsu = wsu_f.rearrange("p (k m) -> p k m", k=KH)
                    pu = ps_sm.tile([128, TC], F32, tag="pstb")
                    for k in range(KH):
                        nc.tensor.matmul(pu, wsu[:, k, :], h2T_ownb[:, k, :],
                                         start=(k == 0), stop=(k == KH - 1))
                    pg = ps_sm.tile([128, TC], F32, tag="pstb")
                    for k in range(KH):
                        nc.tensor.matmul(pg, wsg[:, k, :], h2T_ownb[:, k, :],
                                         start=(k == 0), stop=(k == KH - 1))
                    u_bf = shp.tile([128, TC], BF16, tag="ubf")
                    nc.scalar.activation(u_bf, pu, AF.Copy)
                    g_bf = shp.tile([128, TC], BF16, tag="gbf")
                    nc.scalar.activation(g_bf, pg, AF.Silu)
                    nc.vector.tensor_mul(acts_s[:, i, :], g_bf, u_bf)
                wsd_sb = shp.tile([128, IS // 128, H], BF16, tag="wsd",
                                  bufs=1)
                nc.sync.dma_start(
                    out=wsd_sb,
                    in_=bass.AP(tensor=wsd_pk, offset=0,
                                ap=[[H, 128], [128 * H, IS // 128], [1, H]]))
                for n4 in range(H // 512):
                    psh = ps_big.tile([TC, 512], F32, tag="mm512")
                    for i in range(IS // 128):
                        nc.tensor.matmul(
                            psh, acts_s[:, i, :],
                            wsd_sb[:, i, n4 * 512:(n4 + 1) * 512],
                            start=(i == 0), stop=(i == IS // 128 - 1))
                    nc.vector.tensor_copy(shared_sb[:, n4 * 512:(n4 + 1) * 512],
                                          psh)

            # ================= MOE EXPERTS (group tokens, fp8 W-comp) ======
            with tc.tile_pool(name="mp", bufs=2) as mp, \
                 tc.tile_pool(name="wgup", bufs=3) as wgup:
                h2g = []
                h2g16 = []
                for kp in range(8):
                    t_ = mp.tile([128, 2, TG], F8, tag=f"h2g{kp}", bufs=1,
                                 name=f"h2g{kp}")
                    for j in range(2):
                        nc.scalar.dma_start(
                            out=t_[:, j, :],
                            in_=bass.AP(
                                tensor=hag_out.tensor,
                                offset=hag_out.offset + (kp * 256 + j * 128) * TC,
                                ap=[[TC, 128], [H * TC, NTG], [1, TC]]))
                    h2g.append(t_)
                    t16 = mp.tile([128, 2, TG], F8, tag=f"h2g16_{kp}", bufs=1,
                                  name=f"h2g16_{kp}")
                    nc.vector.tensor_scalar_mul(
                        t16.rearrange("p j t -> p (j t)"),
                        t_.rearrange("p j t -> p (j t)"), c16_sb)
                    h2g16.append(t16)

                # reconstruct per-expert broadcast router weights from the
                # fp8 hi/lo rows carried in the h2 AG payload
                whi8 = mp.tile([E, NTG, TC], F8, tag="whi8", bufs=1)
                nc.scalar.dma_start(
                    out=whi8,
                    in_=bass.AP(tensor=wag_out.tensor,
                                offset=wag_out.offset,
                                ap=[[TC, E], [32 * TC, NTG], [1, TC]]))
                wlo8 = mp.tile([E, NTG, TC], F8, tag="wlo8", bufs=1)
                nc.scalar.dma_start(
                    out=wlo8,
                    in_=bass.AP(tensor=wag_out.tensor,
                                offset=wag_out.offset + E * TC,
                                ap=[[TC, E], [32 * TC, NTG], [1, TC]]))
                w_f = mp.tile([E, TG], BF16, tag="wf", bufs=1)
                nc.scalar.activation(w_f, whi8.rearrange("p c t -> p (c t)"),
                                     AF.Copy)
                w_lo_b = mp.tile([E, TG], BF16, tag="wlob", bufs=1)
                nc.scalar.activation(w_lo_b,
                                     wlo8.rearrange("p c t -> p (c t)"),
                                     AF.Copy, scale=1.0 / 16.0)
                nc.vector.tensor_add(w_f, w_f, w_lo_b)
                w_bc = []
                for ei in range(2):
                    pw = ps_big.tile([128, 512], F32, tag="mm512")
                    nc.tensor.matmul(pw, esel_sb[:, ei, :], w_f, start=True,
                                     stop=True)
                    t_ = mp.tile([128, TG], BF16, tag=f"wbc{ei}", bufs=1,
                                 name=f"wbc{ei}")
                    nc.vector.tensor_copy(t_, pw)
                    w_bc.append(t_)


                # gu with W-compensation; acts -> a8 (+ a16 = act/16)
                a8 = [mp.tile([128, I // 128, TG], F8, tag=f"a8_{e}", bufs=1,
                              name=f"a8_{e}") for e in range(2)]
                a16 = [mp.tile([128, I // 128, TG], F8, tag=f"a16_{e}",
                               bufs=1, name=f"a16_{e}") for e in range(2)]
                wdlo = [mp.tile([128, 16384], F8, tag=f"wdlo{e}", bufs=1,
                                name=f"wdlo{e}") for e in range(2)]
                for e in range(2):
                    if e == 1:
                        for e2 in range(2):
                            nc.scalar.dma_start(out=wdlo[e2],
                                                in_=wd_pk[e2, 1, :, :])
                    for i in range(I // 128):
                        whi = wgup.tile([128, 4096], F8, tag="whi")
                        nc.sync.dma_start(out=whi, in_=wgu_pk[e, 0, i, :, :])
                        wlo = wgup.tile([128, 4096], F8, tag="wlo")
                        nc.sync.dma_start(out=wlo, in_=wgu_pk[e, 1, i, :, :])
                        whiv = whi.rearrange("p (kp j mt m) -> p kp j mt m",
                                             kp=8, j=2, mt=2)
                        wlov = wlo.rearrange("p (kp j mt m) -> p kp j mt m",
                                             kp=8, j=2, mt=2)
                        pu = ps_big.tile([128, 512], F32, tag="mm512")
                        pg = ps_big.tile([128, 512], F32, tag="mm512")
                        for mt, pp in ((1, pu), (0, pg)):
                            for half in range(2):
                                sl = slice(half * 256, (half + 1) * 256)
                                for kp in range(8):
                                    nc.tensor.matmul(
                                        pp[:, half * 256:(half + 1) * 256],
                                        whiv[:, kp, :, mt, :],
                                        h2g[kp][:, :, sl],
                                        start=(kp == 0), stop=False,
                                        perf_mode=DR)
                                for kp in range(8):
                                    nc.tensor.matmul(
                                        pp[:, half * 256:(half + 1) * 256],
                                        wlov[:, kp, :, mt, :],
                                        h2g16[kp][:, :, sl],
                                        start=False, stop=(kp == 7),
                                        perf_mode=DR)
                        u_bf = sb.tile([128, TG], BF16, tag="ubfm")
                        nc.scalar.activation(u_bf, pu, AF.Copy)
                        g_bf = sb.tile([128, TG], BF16, tag="gbfm")
                        nc.scalar.activation(g_bf, pg, AF.Silu)
                        nc.vector.tensor_mul(g_bf, g_bf, u_bf)
                        nc.vector.tensor_mul(g_bf, g_bf, w_bc[e])
                        nc.scalar.activation(a8[e][:, i, :], g_bf, AF.Copy)
                        nc.scalar.activation(a16[e][:, i, :], g_bf, AF.Copy,
                                             scale=1.0 / 16.0)

                # down-proj with W-comp; two H-halves, RS each
                wdhiv = [wdhi[e].rearrange("p (kp j n) -> p kp j n",
                                           kp=4, j=2) for e in range(2)]
                wdlov = [wdlo[e].rearrange("p (kp j n) -> p kp j n",
                                           kp=4, j=2) for e in range(2)]
                for q in range(2):
                    for t in range(NTG):
                        moe_bf = sb.tile([TC, H // 2], BF16, tag="moebf")
                        for n4 in range(2):
                            pd = ps_big.tile([TC, 512], F32, tag="mm512")
                            for half in range(2):
                                n0 = q * 1024 + n4 * 512 + half * 256
                                psl = pd[:, half * 256:(half + 1) * 256]
                                for e in range(2):
                                    for kp in range(4):
                                        nc.tensor.matmul(
                                            psl,
                                            a8[e][:, 2 * kp:2 * kp + 2,
                                                  t * TC:(t + 1) * TC],
                                            wdhiv[e][:, kp, :, n0:n0 + 256],
                                            start=(e == 0 and kp == 0),
                                            stop=False, perf_mode=DR)
                                for e in range(2):
                                    for kp in range(4):
                                        nc.tensor.matmul(
                                            psl,
                                            a16[e][:, 2 * kp:2 * kp + 2,
                                                   t * TC:(t + 1) * TC],
                                            wdlov[e][:, kp, :, n0:n0 + 256],
                                            start=False,
                                            stop=(e == 1 and kp == 3),
                                            perf_mode=DR)
                            nc.vector.tensor_copy(
                                moe_bf[:, n4 * 512:(n4 + 1) * 512], pd)
                        nc.scalar.dma_start(
                            out=moe_in[q][t * TC:(t + 1) * TC, :],
                            in_=moe_bf)
                    nc.gpsimd.collective_compute(
                        "ReduceScatter", ALU.add, replica_groups=rg4,
                        ins=[moe_in[q].opt()], outs=[moe_out[q].opt()])

            # ================= FINAL =================
            pre_sb = sb.tile([TC, H], F32, tag="pre", bufs=1)
            nc.vector.tensor_add(pre_sb, shared_sb, x2_sb)
            for q in range(2):
                sl = slice(q * 1024, (q + 1) * 1024)
                mo_bf = sb.tile([TC, H // 2], BF16, tag="mobf")
                nc.scalar.dma_start(out=mo_bf, in_=moe_out[q][:, :])
                mo_f = sb.tile([TC, H // 2], F32, tag="mof")
                nc.vector.tensor_copy(mo_f, mo_bf)
                nc.vector.tensor_add(mo_f, mo_f, pre_sb[:, sl])
                nc.sync.dma_start(out=out_chunk[:, sl], in_=mo_f)

    nc.compile()
    return nc


def _prep_inputs(hidden_states, w_ln1, w_ln2, wqkv, q_norm_w, k_norm_w, wo,
                 w_router, w_gu, w_d, ws_gu, ws_d, positions):
    import ml_dtypes
    bf = ml_dtypes.bfloat16
    f8 = ml_dtypes.float8_e4m3fn

    x = np.asarray(hidden_states, np.float32).reshape(T, H)
    w_ln1 = np.asarray(w_ln1, np.float32)
    w_ln2 = np.asarray(w_ln2, np.float32)
    wqkv_e = np.asarray(wqkv, np.float32) * w_ln1[:, None]
    wo_f = np.asarray(wo, np.float32)
    wgu_e = np.asarray(w_gu, np.float32) * w_ln2[None, :, None]
    wd_f = np.asarray(w_d, np.float32)
    wsgu_e = np.asarray(ws_gu, np.float32) * w_ln2[:, None]
    wsd_f = np.asarray(ws_d, np.float32)
    wrT_e = np.ascontiguousarray(
        (np.asarray(w_router, np.float32) * w_ln2[None, :]).T)

    v = np.mean(np.square(x), axis=-1, keepdims=True)
    h1 = (x / np.sqrt(v + EPS)).astype(bf)
    h1T_pack = np.ascontiguousarray(h1.T.reshape(KH, 128, T))

    pos = np.asarray(positions).astype(np.float64)
    inv_freq = 1.0 / (10000.0 ** (np.arange(0, DH, 2, dtype=np.float64) / DH))
    freqs = pos[:, None] * inv_freq[None, :]
    cos = np.cos(freqs).astype(np.float32)
    sin = np.sin(freqs).astype(np.float32)
    qw = np.asarray(q_norm_w, np.float32)
    kw = np.asarray(k_norm_w, np.float32)

    def rope_tab(w):
        tabs = np.stack([cos * w[None, :64], sin * w[None, 64:],
                         cos * w[None, 64:], sin * w[None, :64]],
                        axis=1).astype(np.float32)
        # [T,4,64] -> [128p, NT,4,64] flattened per partition
        return np.ascontiguousarray(
            tabs.reshape(NT, 128, 4, 64).transpose(1, 0, 2, 3)
            .reshape(128, NT * 4 * 64))

    rq = rope_tab(qw)
    rk = rope_tab(kw)

    a = np.arange(128)
    tri = np.where(a[None, :] >= a[:, None], 0.0, NEG).astype(np.float32)

    def q8r(w):
        return w.astype(f8).astype(np.float32)

    def pack_gu8(w2d):
        # hi/lo fp8 packs [2, I/128, 128, 4096]:
        # [i][p][kp*512 + j*256 + mt*128 + m],
        # element = W[kp*256 + j*128 + p, (mt? I+i*128+m : i*128+m)]
        I_ = w2d.shape[1] // 2
        ni = I_ // 128
        hi = np.empty((ni, 128, 4096), np.float32)
        lo = np.empty((ni, 128, 4096), np.float32)
        w_hi = q8r(w2d)
        w_lo = 16.0 * (w2d - w_hi)
        for src, dst in ((w_hi, hi), (w_lo, lo)):
            w4 = src.reshape(8, 2, 128, 2 * I_)
            for i in range(ni):
                g = w4[:, :, :, i * 128:(i + 1) * 128]
                u = w4[:, :, :, I_ + i * 128:I_ + (i + 1) * 128]
                blk = np.stack([g, u], axis=3)
                dst[i] = blk.transpose(2, 0, 1, 3, 4).reshape(128, 4096)
        return np.stack([hi, lo]).astype(f8)

    def pack_d8(w2d):
        # hi/lo fp8 packs [2, 128, 16384]: [p][kp*2*H + j*H + n],
        # element = W[kp*256 + j*128 + p, n]
        I_ = w2d.shape[0]
        w_hi = q8r(w2d)
        w_lo = 16.0 * (w2d - w_hi)
        out = []
        for src in (w_hi, w_lo):
            w4 = src.reshape(I_ // 256, 2, 128, H)
            out.append(w4.transpose(2, 0, 1, 3).reshape(128, (I_ // 256) * 2 * H))
        return np.stack(out).astype(f8)

    # shared expert bf16 packs
    # wsgu_pk [2*IS/128 m-tiles (g0,u0,g1,u1...), 128 p, KH, 128 m]
    ws_m = np.empty((2 * IS // 128, 128, KH, 128), np.float32)
    for i in range(IS // 128):
        g = wsgu_e[:, i * 128:(i + 1) * 128]       # [H, 128]
        u = wsgu_e[:, IS + i * 128:IS + (i + 1) * 128]
        # -> [p][ks][m]
        ws_m[2 * i] = g.reshape(KH, 128, 128).transpose(1, 0, 2)
        ws_m[2 * i + 1] = u.reshape(KH, 128, 128).transpose(1, 0, 2)
    wsgu_pack = ws_m.astype(bf)
    wsd_pack = np.ascontiguousarray(
        wsd_f.reshape(IS // 128, 128, H)).astype(bf)

    in_maps = []
    for c in range(N_CORES):
        gkv = c // 2
        qcols = np.arange(2 * c * DH, (2 * c + 2) * DH)
        kcols = np.arange(NH * DH + gkv * DH, NH * DH + (gkv + 1) * DH)
        vcols = np.arange(NH * DH + NKV * DH + gkv * DH,
                          NH * DH + NKV * DH + (gkv + 1) * DH)
        cols = np.concatenate([qcols, kcols, vcols])
        wq_sl = np.ascontiguousarray(
            wqkv_e[:, cols].astype(bf).reshape(KH, 128, 512))
        wo_slc = np.ascontiguousarray(
            wo_f[2 * c * DH:(2 * c + 2) * DH, :].astype(bf)
            .reshape(2, 128, H))
        e0 = 2 * (c % 4)
        wgu_pack = np.stack([pack_gu8(wgu_e[e0]), pack_gu8(wgu_e[e0 + 1])])
        wd_pack = np.stack([pack_d8(wd_f[e0]), pack_d8(wd_f[e0 + 1])])
        es = np.zeros((2, E, 128), np.float32)
        es[0, e0, :] = 1.0
        es[1, e0 + 1, :] = 1.0
        es = np.ascontiguousarray(es.transpose(1, 0, 2))
        in_maps.append({
            "h1T_in": h1T_pack,
            "wqkv_sl": wq_sl,
            "wo_sl": wo_slc,
            "x_own": np.ascontiguousarray(x[c * TC:(c + 1) * TC]),
            "rope_q": rq,
            "rope_k": rk,
            "trimask": tri,
            "wrT": np.ascontiguousarray(
                wrT_e.reshape(KH, 128, E)).astype(np.float32),
            "wgu_pk": wgu_pack,
            "wd_pk": wd_pack,
            "wsgu_pk": wsgu_pack,
            "wsd_pk": wsd_pack,
            "esel_bc": es.astype(ml_dtypes.bfloat16),
        })
    return in_maps


def kernel(**inputs):
    import os
    if "nc" not in _cache:
        _cache["nc"] = build_nc()
    nc = _cache["nc"]
    in_maps = _prep_inputs(**inputs)
    trace = bool(int(os.environ.get("KERNEL_TRACE", "0")))
    res = run_bass_kernel_spmd(nc, in_maps, core_ids=list(range(N_CORES)),
                               trace=trace)
    _cache["last_result"] = res
    out = np.concatenate(
        [res.results[c]["out_chunk"] for c in range(N_CORES)], axis=0)
    return out.reshape(1, T, H).astype(np.float32)


if __name__ == "__main__":
    import reference
    inp = {k: np.asarray(v) for k, v in reference.setup_inputs().items()}
    got = kernel(**inp)
    exp = np.asarray(reference.reference(**reference.setup_inputs()))
    denom = np.abs(exp).max()
    err = np.abs(got - exp).max() / denom
    print("abs max:", denom, "rel err:", err)
